# revision 43
# baseline (speedup 1.0000x reference)
"""DiscFace AM-softmax loss kernel for 8 TRN2 NeuronCores (v2).

Strategy (tensor-parallel over classes):
  - id_agent/b sharded row-wise: core k owns classes [k*12500, (k+1)*12500),
    padded to 12800 rows with zeros (pad logits == 0 exactly; the constant
    8*300 = 2400 is subtracted during the final correction).
  - Host uploads layout-transformed shards only (no arithmetic):
      iat  [D, CSH]  bf16  class-major transpose of the ia shard (for matmul)
      ia   [CSH, D]  bf16  row-major (indirect gathers only)
      b    [CSH, D]  bf16  row-major (indirect gathers only)
      x    [B, D]    bf16  row-major (norms + disc path)
      xt   [D, B]    bf16  transpose of x (matmul lhsT)
  - Per-class norms on device: sq = iat*iat (DVE), column-sum via a
    ones-matmul on the PE (output replicated across all 128 partitions),
    s_rep = exp(-0.5*ln(ss + eps) + ln 8) on ACT.  Weights w8 = fp8(iat *
    s_rep) via one DVE pass.  Zero pad rows stay exactly zero.
  - Logits: out[b, c] = xT.T @ w8 with fp8 DoubleRow matmuls (2 k-tiles of
    128 per instruction), accumulating d=512 in two DoubleRow steps.
    The batch-side normalization 64/(8*||x_b||) is folded into the ACT
    exp as a per-partition scale; exp+Z-partial-sum fused via accum_out.
  - Margin on the target class via the exact-scalar correction
    Z += exp(64*st - 22.4) - exp(64*st), st from the (bf16) gather path.
  - Two AllReduces ([128,16] disc payload early, [128,8] Z partials at the
    end), then every core finishes the focal + disc loss math.
"""

import os
import sys

import numpy as np

sys.path.insert(0, "/opt/trn_rl_repo")

import ml_dtypes  # noqa: E402

from concourse import bass, mybir, tile  # noqa: E402
from concourse.bass_utils import run_bass_kernel_spmd  # noqa: E402

B, D, C = 1024, 512, 100000
NCORES = 8
CPER = C // NCORES          # 12500 real classes per core
CSH = 12800                 # padded shard rows
NPAD_TOTAL = float(NCORES * (CSH - CPER))   # 2400 pad contributions to Z
BT = B // 128               # 8 batch tiles
NDB = D // 128              # 4 contraction blocks of 128
NCH = CSH // 512            # 25 class chunks of 512
# pieces: groups of chunks sharing one psum tile (3 banks = 1536 cols)
PIECES = [(p * 3, 3) for p in range(8)] + [(24, 1)]
NPIECE = len(PIECES)        # 9

SCALE = 64.0
MARGIN = 0.35
LAMBDA = 0.4
SM = SCALE * MARGIN         # 22.4
WSCL = 8.0                  # weight pre-scale: w = ia * (WSCL/||ia||)
LOG_WSCL = float(np.log(WSCL))   # also = ln(SCALE/WSCL) for the x side
LOG_BCLIP = float(np.log(0.05))
SS_EPS = 1e-30

F32 = mybir.dt.float32
BF16 = mybir.dt.bfloat16
FP8 = mybir.dt.float8e4
I32 = mybir.dt.int32
AF = mybir.ActivationFunctionType
ALU = mybir.AluOpType
AX = mybir.AxisListType
DR = mybir.MatmulPerfMode.DoubleRow


# The TRN2 TPB instruction encoding has exactly ONE semaphore-wait slot;
# move extra waits onto same-engine NoOps after tile scheduling.
_NO_SPLIT_CLASSES = ("InstISA", "InstCall")


def split_multi_waits(nc):
    n_nops = 0
    for f in nc.m.functions:
        for bb in f.blocks:
            new_insts = []
            for inst in bb.instructions:
                si = inst.sync_info
                cls = type(inst).__name__
                zero_wait = (
                    cls != "InstISA"
                    and (hasattr(inst, "isa_opcode") or cls == "InstDmaTransposeAnt")
                )
                keep = 0 if zero_wait else 1
                if (
                    si is not None
                    and len(si.on_wait) > keep
                    and cls not in _NO_SPLIT_CLASSES
                ):
                    split = si.on_wait[:-keep] if keep else list(si.on_wait)
                    for w in split:
                        nop = mybir.InstNoOp(
                            name=nc.get_next_instruction_name(),
                            sync_info=mybir.SyncInfo(on_wait=[w], on_update=[]),
                            bass_nofuse=True,
                            engine=inst.engine,
                        )
                        nc.inst_map[nop.name] = nop
                        new_insts.append(nop)
                        n_nops += 1
                    inst.sync_info = mybir.SyncInfo(
                        on_wait=list(si.on_wait[-keep:]) if keep else [],
                        on_update=list(si.on_update),
                    )
                new_insts.append(inst)
            bb.instructions = new_insts
    return n_nops


def build_bass():
    nc = bass.Bass(trn_type="TRN2", num_devices=NCORES)

    x_d = nc.declare_dram_parameter("xbf", [B, D], BF16, isOutput=False)
    xt_d = nc.declare_dram_parameter("xtbf", [D, B], BF16, isOutput=False)
    iat_d = nc.declare_dram_parameter("iatbf", [D, CSH], BF16, isOutput=False)
    ia_d = nc.declare_dram_parameter("iabf", [CSH, D], BF16, isOutput=False)
    bsh_d = nc.declare_dram_parameter("bbf", [CSH, D], BF16, isOutput=False)
    toff_d = nc.declare_dram_parameter("toff", [128, BT], I32, isOutput=False)
    tmask_d = nc.declare_dram_parameter("tmask", [128, BT], F32, isOutput=False)
    out_d = nc.declare_dram_parameter("out", [1], F32, isOutput=True)

    ccin1 = nc.dram_tensor("ccin1", [128, 16], F32)
    ccout1 = nc.dram_tensor("ccout1", [128, 16], F32, addr_space="Shared")
    ccin2 = nc.dram_tensor("ccin2", [128, BT], F32)
    ccout2 = nc.dram_tensor("ccout2", [128, BT], F32, addr_space="Shared")
    ccin3 = nc.dram_tensor("ccin3", [128, BT], F32)
    ccout3 = nc.dram_tensor("ccout3", [128, BT], F32, addr_space="Shared")

    # Register const APs for the nonzero activation biases we use.
    for v in (LOG_WSCL, LOG_BCLIP, -SM, SS_EPS):
        t = nc.alloc_sbuf_tensor(f"const-f32-{v}", [128, 1], F32)
        nc.gpsimd.memset(t.ap(), v)
        nc.const_aps.aps[(F32, v)] = t.ap()
    nc.all_engine_barrier()

    with tile.TileContext(nc) as tc:
        with (
            tc.tile_pool(name="persist", bufs=1) as pp,
            tc.tile_pool(name="iach", bufs=12) as ia_pool,
            tc.tile_pool(name="sq", bufs=3) as sq_pool,
            tc.tile_pool(name="wbf", bufs=3) as wb_pool,
            tc.tile_pool(name="srep", bufs=5) as sr_pool,
            tc.tile_pool(name="lnb", bufs=3) as ln_pool,
            tc.tile_pool(name="dump", bufs=3) as dump_pool,
            tc.tile_pool(name="work", bufs=3) as w_pool,
            tc.tile_pool(name="small", bufs=2) as s_pool,
            tc.tile_pool(name="psmain", bufs=2, space="PSUM") as ps_main,
            tc.tile_pool(name="psss", bufs=2, space="PSUM") as ps_ss,
        ):
            # ---------------- persistent tiles ----------------
            xbf3 = pp.tile([128, BT, D], BF16, tag="xbf3")      # row-major x
            xn3 = pp.tile([128, BT, D], BF16, tag="xn3")        # normalized x
            xTbf = pp.tile([128, NDB, B], BF16, tag="xTbf")
            xT8 = pp.tile([128, NDB, B], FP8, tag="xT8")
            ia8 = pp.tile([128, NDB, CSH], FP8, tag="ia8")      # scaled weights
            ssx = pp.tile([128, BT], F32, tag="ssx")
            lbx = pp.tile([128, BT], F32, tag="lbx")
            xinv = pp.tile([128, BT], F32, tag="xinv")          # 1/||x||
            xscl = pp.tile([128, BT], F32, tag="xscl")          # 8/||x||
            zp2d = pp.tile([128, BT, NPIECE], F32, tag="zp2d")  # exp partials
            payload1 = pp.tile([128, 16], F32, tag="payload1")
            payload2 = pp.tile([128, BT], F32, tag="payload2")
            allred1 = pp.tile([128, 16], F32, tag="allred1")
            allred2 = pp.tile([128, BT], F32, tag="allred2")
            toffs = pp.tile([128, BT], I32, tag="toffs")
            tmasks = pp.tile([128, BT], F32, tag="tmasks")
            ones_bf = pp.tile([128, 128], BF16, tag="ones_bf")
            ones_f32 = pp.tile([128, 1], F32, tag="ones_f32")
            # disc-path persistents
            ng2 = pp.tile([128, BT], F32, tag="ng2")
            dot8 = pp.tile([128, BT], F32, tag="dot8")
            btn2 = pp.tile([128, BT], F32, tag="btn2")
            rn2 = pp.tile([128, BT], F32, tag="rn2")
            s1_8 = pp.tile([128, BT], F32, tag="s1_8")
            f8 = pp.tile([128, BT], F32, tag="f8")
            lb8 = pp.tile([128, BT], F32, tag="lb8")
            g3 = pp.tile([128, BT, D], BF16, tag="g3")          # gathered ia rows
            btg3 = pp.tile([128, BT, D], BF16, tag="btg3")      # gathered b rows

            nc.vector.memset(ones_bf[:], 1.0)
            nc.vector.memset(ones_f32[:], 1.0)

            # ---------------- phase 0 ----------------
            nc.gpsimd.dma_start(out=toffs[:], in_=toff_d[:])
            nc.gpsimd.dma_start(out=tmasks[:], in_=tmask_d[:])

            # preload the ln/exp table set before the hot loop
            warm = s_pool.tile([128, 1], F32, tag="warm")
            nc.scalar.activation(warm[:], ones_f32[:], AF.Exp)

            # x / xT as two coarse loads on the gpsimd SWDGE queue (async
            # ring transfers) so both HWDGE queues carry only chunk loads.
            nc.gpsimd.dma_start(
                out=xTbf[:], in_=xt_d.rearrange("(dbl p) b -> p dbl b", p=128)
            )
            nc.gpsimd.dma_start(
                out=xbf3[:], in_=x_d.rearrange("(bt p) d -> p bt d", p=128)
            )
            def xpath_gen():
                # fp8 cast of xT early in the DVE queue so the first main
                # matmul is gated only by the weight producers.
                nc.vector.tensor_copy(out=xT8[:], in_=xTbf[:])
                yield
                for bt in range(BT):
                    dmp = dump_pool.tile([128, D], F32, tag="dmpf32")
                    nc.vector.scalar_tensor_tensor(
                        out=dmp[:], in0=xbf3[:, bt, :], scalar=1.0,
                        in1=xbf3[:, bt, :], op0=ALU.mult, op1=ALU.mult,
                        accum_out=ssx[:, bt:bt + 1],
                    )
                    if bt % 3 == 2:
                        yield
                nc.vector.tensor_scalar_max(out=ssx[:], in0=ssx[:], scalar1=1e-30)
                nc.scalar.activation(lbx[:], ssx[:], AF.Ln)
                nc.scalar.activation(xinv[:], lbx[:], AF.Exp, scale=-0.5)
                nc.scalar.activation(
                    xscl[:], lbx[:], AF.Exp, scale=-0.5, bias=LOG_WSCL
                )
                yield

            # ---------------- producers: per class chunk ----------------
            iat_r = iat_d.rearrange("(dbl p) c -> p dbl c", p=128)

            chunk_refs = {}

            def produceA(c):
                """Chunk c stage A: load -> square -> colsum (PE ones-mm)
                -> ln/exp scale (replicated).  No DVE op here ever waits
                on ACT, so the DVE queue never serializes on the norm
                chain."""
                c0 = c * 512
                iach = ia_pool.tile([128, NDB, 512], BF16, tag="iach")
                eng = nc.sync if c % 2 == 0 else nc.scalar
                eng.dma_start(out=iach[:], in_=iat_r[:, :, c0:c0 + 512])
                yield
                sq = sq_pool.tile([128, NDB, 512], BF16, tag="sq")
                nc.vector.tensor_tensor(
                    out=sq[:], in0=iach[:], in1=iach[:], op=ALU.mult
                )
                ss_ps = ps_ss.tile([128, 512], F32, tag="ssps")
                for db in range(NDB):
                    nc.tensor.matmul(
                        out=ss_ps[:], lhsT=ones_bf[:], rhs=sq[:, db, :],
                        start=(db == 0), stop=(db == NDB - 1),
                    )
                yield
                lnb = ln_pool.tile([128, 512], F32, tag="lnb")
                nc.scalar.activation(lnb[:], ss_ps[:], AF.Ln, bias=SS_EPS)
                srep = sr_pool.tile([128, 512], BF16, tag="srep")
                nc.scalar.activation(
                    srep[:], lnb[:], AF.Exp, scale=-0.5, bias=LOG_WSCL
                )
                chunk_refs[c] = (iach, srep)
                yield

            def produceB(c):
                """Chunk c stage B (emitted a few chunks behind A): scale
                by the replicated norm factors (DVE 2x bf16) and cast to
                fp8 via a SBUF->SBUF DMA on the gpsimd SWDGE queue."""
                c0 = c * 512
                iach, srep = chunk_refs.pop(c)
                wbf = wb_pool.tile([128, NDB, 512], BF16, tag="wbf")
                nc.vector.tensor_tensor(
                    out=wbf[:], in0=iach[:],
                    in1=srep[:, None, :].broadcast_to([128, NDB, 512]),
                    op=ALU.mult,
                )
                nc.gpsimd.dma_start(out=ia8[:, :, c0:c0 + 512], in_=wbf[:])

            def mm_sweep(p, interleave):
                ch0, nch = PIECES[p]
                w = nch * 512
                for bt in range(BT):
                    ps = ps_main.tile([128, 1536], F32, tag="psmain")
                    for dr in range(2):
                        for j in range(nch):
                            cof = (ch0 + j) * 512
                            nc.tensor.matmul(
                                out=ps[:, j * 512:(j + 1) * 512],
                                lhsT=xT8[:, 2 * dr:2 * dr + 2,
                                         bt * 128:(bt + 1) * 128],
                                rhs=ia8[:, 2 * dr:2 * dr + 2, cof:cof + 512],
                                start=(dr == 0), stop=(dr == 1),
                                perf_mode=DR,
                            )
                    edump = dump_pool.tile([128, 1536], BF16, tag="edump")
                    nc.scalar.activation(
                        edump[:, :w], ps[:, :w], AF.Exp,
                        scale=xscl[:, bt:bt + 1],
                        accum_out=zp2d[:, bt, p:p + 1],
                    )
                    for _ in range(2):
                        next(interleave, None)

            def disc_gen():
                # ---------------- disc-loss gather path ----------------
                # (runs mid-kernel so its gathers/STTs never head-of-line
                # block the producer pipeline)
                for bt in range(BT):
                    nc.vector.tensor_scalar_mul(
                        out=xn3[:, bt, :], in0=xbf3[:, bt, :],
                        scalar1=xinv[:, bt:bt + 1],
                    )
                    yield
                for bt in range(BT):
                    nc.gpsimd.indirect_dma_start(
                        out=g3[:, bt, :], out_offset=None,
                        in_=ia_d[:, :],
                        in_offset=bass.IndirectOffsetOnAxis(
                            ap=toffs[:, bt:bt + 1], axis=0
                        ),
                    )
                    nc.gpsimd.indirect_dma_start(
                        out=btg3[:, bt, :], out_offset=None,
                        in_=bsh_d[:, :],
                        in_offset=bass.IndirectOffsetOnAxis(
                            ap=toffs[:, bt:bt + 1], axis=0
                        ),
                    )
                    dmp = dump_pool.tile([128, D], F32, tag="dmpf32")
                    nc.vector.scalar_tensor_tensor(
                        out=dmp[:], in0=g3[:, bt, :], scalar=1.0,
                        in1=g3[:, bt, :], op0=ALU.mult, op1=ALU.mult,
                        accum_out=ng2[:, bt:bt + 1],
                    )
                    dmp = dump_pool.tile([128, D], F32, tag="dmpf32")
                    nc.vector.scalar_tensor_tensor(
                        out=dmp[:], in0=g3[:, bt, :], scalar=1.0,
                        in1=xn3[:, bt, :], op0=ALU.mult, op1=ALU.mult,
                        accum_out=dot8[:, bt:bt + 1],
                    )
                    dmp = dump_pool.tile([128, D], F32, tag="dmpf32")
                    nc.vector.scalar_tensor_tensor(
                        out=dmp[:], in0=btg3[:, bt, :], scalar=1.0,
                        in1=btg3[:, bt, :], op0=ALU.mult, op1=ALU.mult,
                        accum_out=btn2[:, bt:bt + 1],
                    )
                    yield
                # s1 = 1/||ia_t|| ; f = min(1, 0.05/||bt||)
                nc.vector.tensor_scalar_max(out=ng2[:], in0=ng2[:], scalar1=1e-30)
                nc.vector.tensor_scalar_max(out=btn2[:], in0=btn2[:], scalar1=1e-30)
                nc.scalar.activation(lb8[:], ng2[:], AF.Ln)
                nc.scalar.activation(s1_8[:], lb8[:], AF.Exp, scale=-0.5)
                nc.scalar.activation(lb8[:], btn2[:], AF.Ln)
                nc.scalar.activation(f8[:], lb8[:], AF.Exp, scale=-0.5, bias=LOG_BCLIP)
                nc.vector.tensor_scalar_min(out=f8[:], in0=f8[:], scalar1=1.0)
                yield
                for bt in range(BT):
                    t1 = w_pool.tile([128, D], BF16, tag="wk")
                    nc.vector.scalar_tensor_tensor(
                        out=t1[:], in0=g3[:, bt, :], scalar=s1_8[:, bt:bt + 1],
                        in1=xn3[:, bt, :], op0=ALU.mult, op1=ALU.subtract,
                    )
                    t2 = w_pool.tile([128, D], BF16, tag="wk")
                    dmp = dump_pool.tile([128, D], F32, tag="dmpf32")
                    nc.vector.scalar_tensor_tensor(
                        out=t2[:], in0=btg3[:, bt, :], scalar=f8[:, bt:bt + 1],
                        in1=t1[:], op0=ALU.mult, op1=ALU.add,
                    )
                    nc.vector.scalar_tensor_tensor(
                        out=dmp[:], in0=t2[:], scalar=1.0,
                        in1=t2[:], op0=ALU.mult, op1=ALU.mult,
                        accum_out=rn2[:, bt:bt + 1],
                    )
                    yield
                # rn = sqrt(rn2); st = dot * s1; payload cols 0:8 st, 8:16 rn
                nc.vector.tensor_scalar_max(out=rn2[:], in0=rn2[:], scalar1=1e-30)
                nc.scalar.activation(lb8[:], rn2[:], AF.Ln)
                nc.scalar.activation(lb8[:], lb8[:], AF.Exp, scale=0.5)
                nc.vector.tensor_tensor(
                    out=payload1[:, 8:16], in0=lb8[:], in1=tmasks[:], op=ALU.mult
                )
                nc.vector.tensor_tensor(
                    out=s1_8[:], in0=dot8[:], in1=s1_8[:], op=ALU.mult
                )
                nc.vector.tensor_tensor(
                    out=payload1[:, 0:8], in0=s1_8[:], in1=tmasks[:], op=ALU.mult
                )
                # early all-reduce of the disc-path payload; overlaps the
                # main loop, so e1/e2 are ready before Z lands.
                nc.gpsimd.dma_start(out=ccin1[:], in_=payload1[:])
                nc.gpsimd.collective_compute(
                    "AllReduce", ALU.add,
                    replica_groups=[list(range(NCORES))],
                    ins=[ccin1[:]], outs=[ccout1[:]],
                )
                nc.gpsimd.dma_start(out=allred1[:], in_=ccout1[:])
                e1 = s_pool.tile([128, 8], F32, tag="e1")
                e2 = s_pool.tile([128, 8], F32, tag="e2")
                eref["e1"], eref["e2"] = e1, e2
                nc.scalar.activation(e1[:], allred1[:, 0:8], AF.Exp, scale=SCALE)
                nc.scalar.activation(
                    e2[:], allred1[:, 0:8], AF.Exp, scale=SCALE, bias=-SM
                )
                yield

            eref = {}

            def producer_chain():
                xp = xpath_gen()
                dg = disc_gen()
                for c in range(NCH):
                    for _ in produceA(c):
                        yield
                        if c >= 1:
                            next(xp, None)
                    if c >= 3:
                        produceB(c - 3)
                        yield
                for c in range(NCH - 3, NCH):
                    produceB(c)
                    yield
                for _ in xp:
                    yield
                for _ in dg:
                    yield

            prod = producer_chain()
            # prefill: emit stage-A for chunks 0-11 and stage-B through
            # chunk 8 before the first sweep.
            for _ in range(97):
                next(prod, None)
            payload2b = pp.tile([128, BT], F32, tag="payload2b")
            allred2b = pp.tile([128, BT], F32, tag="allred2b")
            for p in range(NPIECE):
                mm_sweep(p, prod)
                if p == NPIECE - 3:
                    # partial Z all-reduce over finished pieces; its ring
                    # latency hides under the last two piece sweeps.
                    for bt in range(BT):
                        nc.vector.reduce_sum(
                            out=payload2[:, bt:bt + 1],
                            in_=zp2d[:, bt, 0:NPIECE - 2],
                            axis=AX.X,
                        )
                    nc.gpsimd.dma_start(out=ccin2[:], in_=payload2[:])
                    nc.gpsimd.collective_compute(
                        "AllReduce", ALU.add,
                        replica_groups=[list(range(NCORES))],
                        ins=[ccin2[:]], outs=[ccout2[:]],
                    )
                    nc.gpsimd.dma_start(out=allred2[:], in_=ccout2[:])
            for _ in prod:
                pass

            # ---------------- all-reduce the last Z partials ----
            for bt in range(BT):
                nc.vector.reduce_sum(
                    out=payload2b[:, bt:bt + 1],
                    in_=zp2d[:, bt, NPIECE - 2:NPIECE],
                    axis=AX.X,
                )
            nc.gpsimd.dma_start(out=ccin3[:], in_=payload2b[:])
            nc.gpsimd.collective_compute(
                "AllReduce", ALU.add,
                replica_groups=[list(range(NCORES))],
                ins=[ccin3[:]], outs=[ccout3[:]],
            )
            nc.gpsimd.dma_start(out=allred2b[:], in_=ccout3[:])
            nc.vector.tensor_tensor(
                out=allred2[:], in0=allred2[:], in1=allred2b[:], op=ALU.add
            )

            # ---------------- final loss math (identical on all cores) ----
            zsum = allred2[:, 0:8]
            rn8 = allred1[:, 8:16]
            zc = s_pool.tile([128, 8], F32, tag="zc")
            lnz = s_pool.tile([128, 8], F32, tag="lnz")
            nll = s_pool.tile([128, 8], F32, tag="nll")
            nc.vector.tensor_scalar_add(
                out=zc[:], in0=zsum, scalar1=-NPAD_TOTAL
            )
            e1, e2 = eref["e1"], eref["e2"]
            nc.vector.tensor_tensor(out=zc[:], in0=zc[:], in1=e1[:], op=ALU.subtract)
            nc.vector.tensor_tensor(out=zc[:], in0=zc[:], in1=e2[:], op=ALU.add)
            nc.scalar.activation(lnz[:], zc[:], AF.Ln)
            # nll = lnz - 64*st + 22.4
            st8 = allred1[:, 0:8]
            nc.vector.scalar_tensor_tensor(
                out=nll[:], in0=st8, scalar=-SCALE, in1=lnz[:],
                op0=ALU.mult, op1=ALU.add,
            )
            nc.vector.tensor_scalar_add(out=nll[:], in0=nll[:], scalar1=SM)
            red2 = s_pool.tile([128, 2], F32, tag="red2")
            nc.vector.reduce_sum(out=red2[:, 0:1], in_=nll[:], axis=AX.X)
            nc.vector.reduce_sum(out=red2[:, 1:2], in_=rn8, axis=AX.X)
            fin_ps = ps_ss.tile([128, 512], F32, tag="ssps")
            nc.tensor.matmul(
                out=fin_ps[0:1, 0:2], lhsT=ones_f32[:], rhs=red2[:],
                start=True, stop=True,
            )
            fin = s_pool.tile([1, 2], F32, tag="fin")
            nc.vector.tensor_copy(out=fin[:], in_=fin_ps[0:1, 0:2])
            p_t = s_pool.tile([1, 1], F32, tag="p_t")
            nc.scalar.activation(p_t[:], fin[:, 0:1], AF.Exp, scale=-1.0 / B)
            q_t = s_pool.tile([1, 1], F32, tag="q_t")
            nc.vector.tensor_scalar(
                out=q_t[:], in0=p_t[:], scalar1=-1.0, scalar2=1.0,
                op0=ALU.mult, op1=ALU.add,
            )
            nc.vector.tensor_tensor(out=q_t[:], in0=q_t[:], in1=q_t[:], op=ALU.mult)
            lgp = s_pool.tile([1, 1], F32, tag="lgp")
            nc.vector.tensor_scalar_mul(out=lgp[:], in0=fin[:, 0:1], scalar1=1.0 / B)
            nc.vector.tensor_tensor(out=q_t[:], in0=q_t[:], in1=lgp[:], op=ALU.mult)
            rterm = s_pool.tile([1, 1], F32, tag="rterm")
            nc.vector.tensor_scalar_mul(
                out=rterm[:], in0=fin[:, 1:2], scalar1=LAMBDA / B
            )
            nc.vector.tensor_tensor(
                out=q_t[:], in0=q_t[:], in1=rterm[:], op=ALU.add
            )
            nc.gpsimd.dma_start(out=out_d[:], in_=q_t[:])

    n = split_multi_waits(nc)
    print(f"split_multi_waits: inserted {n} wait-nops")
    return nc


_NC_CACHE = {}


def _get_nc():
    if "nc" not in _NC_CACHE:
        _NC_CACHE["nc"] = build_bass()
    return _NC_CACHE["nc"]


def make_in_maps(x, target, id_agent, b):
    x = np.ascontiguousarray(np.asarray(x, dtype=np.float32))
    target = np.asarray(target).astype(np.int64)
    id_agent = np.asarray(id_agent, dtype=np.float32)
    b = np.asarray(b, dtype=np.float32)

    bf = ml_dtypes.bfloat16
    x_bf = x.astype(bf)
    xt_bf = np.ascontiguousarray(x_bf.T)

    in_maps = []
    for k in range(NCORES):
        lo = k * CPER
        ia_k = np.zeros((CSH, D), dtype=bf)
        ia_k[:CPER] = id_agent[lo:lo + CPER].astype(bf)
        b_k = np.zeros((CSH, D), dtype=bf)
        b_k[:CPER] = b[lo:lo + CPER].astype(bf)
        iat_k = np.ascontiguousarray(ia_k.T)
        tloc = np.clip(target - lo, 0, CPER - 1).astype(np.int32)
        owned = ((target >= lo) & (target < lo + CPER)).astype(np.float32)
        toff_k = np.ascontiguousarray(tloc.reshape(BT, 128).T)
        tmask_k = np.ascontiguousarray(owned.reshape(BT, 128).T)
        in_maps.append(
            {
                "xbf": x_bf,
                "xtbf": xt_bf,
                "iatbf": iat_k,
                "iabf": ia_k,
                "bbf": b_k,
                "toff": toff_k,
                "tmask": tmask_k,
            }
        )
    return in_maps


def run(inputs, trace=False, **kw):
    nc = _get_nc()
    in_maps = make_in_maps(**inputs)
    res = run_bass_kernel_spmd(
        nc, in_maps, core_ids=list(range(NCORES)), trace=trace, **kw
    )
    return res


def kernel(x, target, id_agent, b):
    res = run({"x": x, "target": target, "id_agent": id_agent, "b": b})
    return np.asarray(res.results[0]["out"], dtype=np.float32)


# revision 44
# speedup vs baseline: 1.0497x; 1.0497x over previous
"""DiscFace AM-softmax loss kernel for 8 TRN2 NeuronCores (v2).

Strategy (tensor-parallel over classes):
  - id_agent/b sharded row-wise: core k owns classes [k*12500, (k+1)*12500),
    padded to 12800 rows with zeros (pad logits == 0 exactly; the constant
    8*300 = 2400 is subtracted during the final correction).
  - Host uploads layout-transformed shards only (no arithmetic):
      iat  [D, CSH]  bf16  class-major transpose of the ia shard (for matmul)
      ia   [CSH, D]  bf16  row-major (indirect gathers only)
      b    [CSH, D]  bf16  row-major (indirect gathers only)
      x    [B, D]    bf16  row-major (norms + disc path)
      xt   [D, B]    bf16  transpose of x (matmul lhsT)
  - Per-class norms on device: sq = iat*iat (DVE), column-sum via a
    ones-matmul on the PE (output replicated across all 128 partitions),
    s_rep = exp(-0.5*ln(ss + eps) + ln 8) on ACT.  Weights w8 = fp8(iat *
    s_rep) via one DVE pass.  Zero pad rows stay exactly zero.
  - Logits: out[b, c] = xT.T @ w8 with fp8 DoubleRow matmuls (2 k-tiles of
    128 per instruction), accumulating d=512 in two DoubleRow steps.
    The batch-side normalization 64/(8*||x_b||) is folded into the ACT
    exp as a per-partition scale; exp+Z-partial-sum fused via accum_out.
  - Margin on the target class via the exact-scalar correction
    Z += exp(64*st - 22.4) - exp(64*st), st from the (bf16) gather path.
  - Two AllReduces ([128,16] disc payload early, [128,8] Z partials at the
    end), then every core finishes the focal + disc loss math.
"""

import os
import sys

import numpy as np

sys.path.insert(0, "/opt/trn_rl_repo")

import ml_dtypes  # noqa: E402

from concourse import bass, mybir, tile  # noqa: E402
from concourse.bass_utils import run_bass_kernel_spmd  # noqa: E402

B, D, C = 1024, 512, 100000
NCORES = 8
CPER = C // NCORES          # 12500 real classes per core
CSH = 12800                 # padded shard rows
NPAD_TOTAL = float(NCORES * (CSH - CPER))   # 2400 pad contributions to Z
BT = B // 128               # 8 batch tiles
NDB = D // 128              # 4 contraction blocks of 128
NCH = CSH // 512            # 25 class chunks of 512
# pieces: groups of chunks sharing one psum tile (3 banks = 1536 cols)
PIECES = [(p * 3, 3) for p in range(8)] + [(24, 1)]
NPIECE = len(PIECES)        # 9

SCALE = 64.0
MARGIN = 0.35
LAMBDA = 0.4
SM = SCALE * MARGIN         # 22.4
WSCL = 8.0                  # weight pre-scale: w = ia * (WSCL/||ia||)
LOG_WSCL = float(np.log(WSCL))   # also = ln(SCALE/WSCL) for the x side
LOG_BCLIP = float(np.log(0.05))
SS_EPS = 1e-30

F32 = mybir.dt.float32
BF16 = mybir.dt.bfloat16
FP8 = mybir.dt.float8e4
I32 = mybir.dt.int32
AF = mybir.ActivationFunctionType
ALU = mybir.AluOpType
AX = mybir.AxisListType
DR = mybir.MatmulPerfMode.DoubleRow


# The TRN2 TPB instruction encoding has exactly ONE semaphore-wait slot;
# move extra waits onto same-engine NoOps after tile scheduling.
_NO_SPLIT_CLASSES = ("InstISA", "InstCall")


def split_multi_waits(nc):
    n_nops = 0
    for f in nc.m.functions:
        for bb in f.blocks:
            new_insts = []
            for inst in bb.instructions:
                si = inst.sync_info
                cls = type(inst).__name__
                zero_wait = (
                    cls != "InstISA"
                    and (hasattr(inst, "isa_opcode") or cls == "InstDmaTransposeAnt")
                )
                keep = 0 if zero_wait else 1
                if (
                    si is not None
                    and len(si.on_wait) > keep
                    and cls not in _NO_SPLIT_CLASSES
                ):
                    split = si.on_wait[:-keep] if keep else list(si.on_wait)
                    for w in split:
                        nop = mybir.InstNoOp(
                            name=nc.get_next_instruction_name(),
                            sync_info=mybir.SyncInfo(on_wait=[w], on_update=[]),
                            bass_nofuse=True,
                            engine=inst.engine,
                        )
                        nc.inst_map[nop.name] = nop
                        new_insts.append(nop)
                        n_nops += 1
                    inst.sync_info = mybir.SyncInfo(
                        on_wait=list(si.on_wait[-keep:]) if keep else [],
                        on_update=list(si.on_update),
                    )
                new_insts.append(inst)
            bb.instructions = new_insts
    return n_nops


def build_bass():
    nc = bass.Bass(trn_type="TRN2", num_devices=NCORES)

    x_d = nc.declare_dram_parameter("xbf", [B, D], BF16, isOutput=False)
    xt_d = nc.declare_dram_parameter("xtbf", [D, B], BF16, isOutput=False)
    iat_d = nc.declare_dram_parameter("iatbf", [D, CSH], BF16, isOutput=False)
    ia_d = nc.declare_dram_parameter("iabf", [CSH, D], BF16, isOutput=False)
    bsh_d = nc.declare_dram_parameter("bbf", [CSH, D], BF16, isOutput=False)
    toff_d = nc.declare_dram_parameter("toff", [128, BT], I32, isOutput=False)
    tmask_d = nc.declare_dram_parameter("tmask", [128, BT], F32, isOutput=False)
    out_d = nc.declare_dram_parameter("out", [1], F32, isOutput=True)

    ccin1 = nc.dram_tensor("ccin1", [128, 16], F32)
    ccout1 = nc.dram_tensor("ccout1", [128, 16], F32, addr_space="Shared")
    ccin2 = nc.dram_tensor("ccin2", [128, BT], F32)
    ccout2 = nc.dram_tensor("ccout2", [128, BT], F32, addr_space="Shared")
    ccin3 = nc.dram_tensor("ccin3", [128, BT], F32)
    ccout3 = nc.dram_tensor("ccout3", [128, BT], F32, addr_space="Shared")

    # Register const APs for the nonzero activation biases we use.
    for v in (LOG_WSCL, LOG_BCLIP, -SM, SS_EPS):
        t = nc.alloc_sbuf_tensor(f"const-f32-{v}", [128, 1], F32)
        nc.gpsimd.memset(t.ap(), v)
        nc.const_aps.aps[(F32, v)] = t.ap()
    nc.all_engine_barrier()

    with tile.TileContext(nc) as tc:
        with (
            tc.tile_pool(name="persist", bufs=1) as pp,
            tc.tile_pool(name="iach", bufs=12) as ia_pool,
            tc.tile_pool(name="sq", bufs=3) as sq_pool,
            tc.tile_pool(name="wbf", bufs=3) as wb_pool,
            tc.tile_pool(name="srep", bufs=5) as sr_pool,
            tc.tile_pool(name="lnb", bufs=3) as ln_pool,
            tc.tile_pool(name="dump", bufs=3) as dump_pool,
            tc.tile_pool(name="work", bufs=3) as w_pool,
            tc.tile_pool(name="small", bufs=2) as s_pool,
            tc.tile_pool(name="psmain", bufs=2, space="PSUM") as ps_main,
            tc.tile_pool(name="psss", bufs=2, space="PSUM") as ps_ss,
        ):
            # ---------------- persistent tiles ----------------
            xbf3 = pp.tile([128, BT, D], BF16, tag="xbf3")      # row-major x
            xn3 = pp.tile([128, BT, D], BF16, tag="xn3")        # normalized x
            xTbf = pp.tile([128, NDB, B], BF16, tag="xTbf")
            xT8 = pp.tile([128, NDB, B], FP8, tag="xT8")
            ia8 = pp.tile([128, NDB, CSH], FP8, tag="ia8")      # scaled weights
            ssx = pp.tile([128, BT], F32, tag="ssx")
            lbx = pp.tile([128, BT], F32, tag="lbx")
            xinv = pp.tile([128, BT], F32, tag="xinv")          # 1/||x||
            xscl = pp.tile([128, BT], F32, tag="xscl")          # 8/||x||
            zp2d = pp.tile([128, BT, NPIECE], F32, tag="zp2d")  # exp partials
            payload1 = pp.tile([128, 16], F32, tag="payload1")
            payload2 = pp.tile([128, BT], F32, tag="payload2")
            allred1 = pp.tile([128, 16], F32, tag="allred1")
            allred2 = pp.tile([128, BT], F32, tag="allred2")
            toffs = pp.tile([128, BT], I32, tag="toffs")
            tmasks = pp.tile([128, BT], F32, tag="tmasks")
            ones_bf = pp.tile([128, 128], BF16, tag="ones_bf")
            ones_f32 = pp.tile([128, 1], F32, tag="ones_f32")
            # disc-path persistents
            ng2 = pp.tile([128, BT], F32, tag="ng2")
            dot8 = pp.tile([128, BT], F32, tag="dot8")
            btn2 = pp.tile([128, BT], F32, tag="btn2")
            rn2 = pp.tile([128, BT], F32, tag="rn2")
            s1_8 = pp.tile([128, BT], F32, tag="s1_8")
            f8 = pp.tile([128, BT], F32, tag="f8")
            lb8 = pp.tile([128, BT], F32, tag="lb8")
            g3 = pp.tile([128, BT, D], BF16, tag="g3")          # gathered ia rows
            btg3 = pp.tile([128, BT, D], BF16, tag="btg3")      # gathered b rows

            nc.vector.memset(ones_bf[:], 1.0)
            nc.vector.memset(ones_f32[:], 1.0)

            # ---------------- phase 0 ----------------
            nc.gpsimd.dma_start(out=toffs[:], in_=toff_d[:])
            nc.gpsimd.dma_start(out=tmasks[:], in_=tmask_d[:])

            # preload the ln/exp table set before the hot loop
            warm = s_pool.tile([128, 1], F32, tag="warm")
            nc.scalar.activation(warm[:], ones_f32[:], AF.Exp)

            # x / xT as two coarse loads on the gpsimd SWDGE queue (async
            # ring transfers) so both HWDGE queues carry only chunk loads.
            nc.gpsimd.dma_start(
                out=xTbf[:], in_=xt_d.rearrange("(dbl p) b -> p dbl b", p=128)
            )
            nc.gpsimd.dma_start(
                out=xbf3[:], in_=x_d.rearrange("(bt p) d -> p bt d", p=128)
            )
            def xpath_gen():
                # fp8 cast of xT early in the DVE queue so the first main
                # matmul is gated only by the weight producers.
                nc.vector.tensor_copy(out=xT8[:], in_=xTbf[:])
                yield
                for bt in range(BT):
                    dmp = dump_pool.tile([128, D], F32, tag="dmpf32")
                    nc.vector.scalar_tensor_tensor(
                        out=dmp[:], in0=xbf3[:, bt, :], scalar=1.0,
                        in1=xbf3[:, bt, :], op0=ALU.mult, op1=ALU.mult,
                        accum_out=ssx[:, bt:bt + 1],
                    )
                    if bt % 3 == 2:
                        yield
                nc.vector.tensor_scalar_max(out=ssx[:], in0=ssx[:], scalar1=1e-30)
                nc.scalar.activation(lbx[:], ssx[:], AF.Ln)
                nc.scalar.activation(xinv[:], lbx[:], AF.Exp, scale=-0.5)
                nc.scalar.activation(
                    xscl[:], lbx[:], AF.Exp, scale=-0.5, bias=LOG_WSCL
                )
                yield

            # ---------------- producers: per class chunk ----------------
            iat_r = iat_d.rearrange("(dbl p) c -> p dbl c", p=128)

            chunk_refs = {}

            def produceA(c):
                """Chunk c stage A: load -> square -> colsum (PE ones-mm)
                -> ln/exp scale (replicated).  No DVE op here ever waits
                on ACT, so the DVE queue never serializes on the norm
                chain."""
                c0 = c * 512
                iach = ia_pool.tile([128, NDB, 512], BF16, tag="iach")
                eng = nc.sync if c % 2 == 0 else nc.scalar
                eng.dma_start(out=iach[:], in_=iat_r[:, :, c0:c0 + 512])
                yield
                sq = sq_pool.tile([128, NDB, 512], BF16, tag="sq")
                nc.vector.tensor_tensor(
                    out=sq[:], in0=iach[:], in1=iach[:], op=ALU.mult
                )
                ss_ps = ps_ss.tile([128, 512], F32, tag="ssps")
                for db in range(NDB):
                    nc.tensor.matmul(
                        out=ss_ps[:], lhsT=ones_bf[:], rhs=sq[:, db, :],
                        start=(db == 0), stop=(db == NDB - 1),
                    )
                yield
                lnb = ln_pool.tile([128, 512], F32, tag="lnb")
                nc.scalar.activation(lnb[:], ss_ps[:], AF.Ln, bias=SS_EPS)
                srep = sr_pool.tile([128, 512], BF16, tag="srep")
                nc.scalar.activation(
                    srep[:], lnb[:], AF.Exp, scale=-0.5, bias=LOG_WSCL
                )
                chunk_refs[c] = (iach, srep)
                yield

            def produceB(c):
                """Chunk c stage B (emitted a few chunks behind A): scale
                by the replicated norm factors (DVE 2x bf16) and cast to
                fp8 via a SBUF->SBUF DMA on the gpsimd SWDGE queue."""
                c0 = c * 512
                iach, srep = chunk_refs.pop(c)
                wbf = wb_pool.tile([128, NDB, 512], BF16, tag="wbf")
                nc.vector.tensor_tensor(
                    out=wbf[:], in0=iach[:],
                    in1=srep[:, None, :].broadcast_to([128, NDB, 512]),
                    op=ALU.mult,
                )
                nc.gpsimd.dma_start(out=ia8[:, :, c0:c0 + 512], in_=wbf[:])

            def mm_sweep(p, interleave):
                ch0, nch = PIECES[p]
                w = nch * 512
                for bt in range(BT):
                    ps = ps_main.tile([128, 1536], F32, tag="psmain")
                    for dr in range(2):
                        for j in range(nch):
                            cof = (ch0 + j) * 512
                            nc.tensor.matmul(
                                out=ps[:, j * 512:(j + 1) * 512],
                                lhsT=xT8[:, 2 * dr:2 * dr + 2,
                                         bt * 128:(bt + 1) * 128],
                                rhs=ia8[:, 2 * dr:2 * dr + 2, cof:cof + 512],
                                start=(dr == 0), stop=(dr == 1),
                                perf_mode=DR,
                            )
                    edump = dump_pool.tile([128, 1536], BF16, tag="edump")
                    nc.scalar.activation(
                        edump[:, :w], ps[:, :w], AF.Exp,
                        scale=xscl[:, bt:bt + 1],
                        accum_out=zp2d[:, bt, p:p + 1],
                    )
                    for _ in range(2):
                        next(interleave, None)

            def disc_gen():
                # ---------------- disc-loss gather path ----------------
                # (runs mid-kernel so its gathers/STTs never head-of-line
                # block the producer pipeline)
                for bt in range(BT):
                    nc.vector.tensor_scalar_mul(
                        out=xn3[:, bt, :], in0=xbf3[:, bt, :],
                        scalar1=xinv[:, bt:bt + 1],
                    )
                    yield
                for bt in range(BT):
                    nc.gpsimd.indirect_dma_start(
                        out=g3[:, bt, :], out_offset=None,
                        in_=ia_d[:, :],
                        in_offset=bass.IndirectOffsetOnAxis(
                            ap=toffs[:, bt:bt + 1], axis=0
                        ),
                    )
                    nc.gpsimd.indirect_dma_start(
                        out=btg3[:, bt, :], out_offset=None,
                        in_=bsh_d[:, :],
                        in_offset=bass.IndirectOffsetOnAxis(
                            ap=toffs[:, bt:bt + 1], axis=0
                        ),
                    )
                    dmp = dump_pool.tile([128, D], F32, tag="dmpf32")
                    nc.vector.scalar_tensor_tensor(
                        out=dmp[:], in0=g3[:, bt, :], scalar=1.0,
                        in1=g3[:, bt, :], op0=ALU.mult, op1=ALU.mult,
                        accum_out=ng2[:, bt:bt + 1],
                    )
                    dmp = dump_pool.tile([128, D], F32, tag="dmpf32")
                    nc.vector.scalar_tensor_tensor(
                        out=dmp[:], in0=g3[:, bt, :], scalar=1.0,
                        in1=xn3[:, bt, :], op0=ALU.mult, op1=ALU.mult,
                        accum_out=dot8[:, bt:bt + 1],
                    )
                    dmp = dump_pool.tile([128, D], F32, tag="dmpf32")
                    nc.vector.scalar_tensor_tensor(
                        out=dmp[:], in0=btg3[:, bt, :], scalar=1.0,
                        in1=btg3[:, bt, :], op0=ALU.mult, op1=ALU.mult,
                        accum_out=btn2[:, bt:bt + 1],
                    )
                    yield
                # s1 = 1/||ia_t|| ; f = min(1, 0.05/||bt||)
                nc.vector.tensor_scalar_max(out=ng2[:], in0=ng2[:], scalar1=1e-30)
                nc.vector.tensor_scalar_max(out=btn2[:], in0=btn2[:], scalar1=1e-30)
                nc.scalar.activation(lb8[:], ng2[:], AF.Ln)
                nc.scalar.activation(s1_8[:], lb8[:], AF.Exp, scale=-0.5)
                nc.scalar.activation(lb8[:], btn2[:], AF.Ln)
                nc.scalar.activation(f8[:], lb8[:], AF.Exp, scale=-0.5, bias=LOG_BCLIP)
                nc.vector.tensor_scalar_min(out=f8[:], in0=f8[:], scalar1=1.0)
                yield
                for bt in range(BT):
                    t1 = w_pool.tile([128, D], BF16, tag="wk")
                    nc.vector.scalar_tensor_tensor(
                        out=t1[:], in0=g3[:, bt, :], scalar=s1_8[:, bt:bt + 1],
                        in1=xn3[:, bt, :], op0=ALU.mult, op1=ALU.subtract,
                    )
                    t2 = w_pool.tile([128, D], BF16, tag="wk")
                    dmp = dump_pool.tile([128, D], F32, tag="dmpf32")
                    nc.vector.scalar_tensor_tensor(
                        out=t2[:], in0=btg3[:, bt, :], scalar=f8[:, bt:bt + 1],
                        in1=t1[:], op0=ALU.mult, op1=ALU.add,
                    )
                    nc.vector.scalar_tensor_tensor(
                        out=dmp[:], in0=t2[:], scalar=1.0,
                        in1=t2[:], op0=ALU.mult, op1=ALU.mult,
                        accum_out=rn2[:, bt:bt + 1],
                    )
                    yield
                # rn = sqrt(rn2); st = dot * s1; payload cols 0:8 st, 8:16 rn
                nc.vector.tensor_scalar_max(out=rn2[:], in0=rn2[:], scalar1=1e-30)
                nc.scalar.activation(lb8[:], rn2[:], AF.Ln)
                nc.scalar.activation(lb8[:], lb8[:], AF.Exp, scale=0.5)
                nc.vector.tensor_tensor(
                    out=payload1[:, 8:16], in0=lb8[:], in1=tmasks[:], op=ALU.mult
                )
                nc.vector.tensor_tensor(
                    out=s1_8[:], in0=dot8[:], in1=s1_8[:], op=ALU.mult
                )
                nc.vector.tensor_tensor(
                    out=payload1[:, 0:8], in0=s1_8[:], in1=tmasks[:], op=ALU.mult
                )
                # early all-reduce of the disc-path payload; overlaps the
                # main loop, so e1/e2 are ready before Z lands.
                nc.gpsimd.dma_start(out=ccin1[:], in_=payload1[:])
                nc.gpsimd.collective_compute(
                    "AllReduce", ALU.add,
                    replica_groups=[list(range(NCORES))],
                    ins=[ccin1[:]], outs=[ccout1[:]],
                )
                nc.gpsimd.dma_start(out=allred1[:], in_=ccout1[:])
                e1 = s_pool.tile([128, 8], F32, tag="e1")
                e2 = s_pool.tile([128, 8], F32, tag="e2")
                eref["e1"], eref["e2"] = e1, e2
                nc.scalar.activation(e1[:], allred1[:, 0:8], AF.Exp, scale=SCALE)
                nc.scalar.activation(
                    e2[:], allred1[:, 0:8], AF.Exp, scale=SCALE, bias=-SM
                )
                yield

            eref = {}

            def producer_chain():
                xp = xpath_gen()
                dg = disc_gen()
                for c in range(NCH):
                    for _ in produceA(c):
                        yield
                        if c >= 1:
                            next(xp, None)
                    if c >= 3:
                        produceB(c - 3)
                        yield
                for c in range(NCH - 3, NCH):
                    produceB(c)
                    yield
                for _ in xp:
                    yield
                for _ in dg:
                    yield

            prod = producer_chain()
            # prefill: emit stage-A for chunks 0-11 and stage-B through
            # chunk 8 before the first sweep.
            for _ in range(45):
                next(prod, None)
            payload2b = pp.tile([128, BT], F32, tag="payload2b")
            allred2b = pp.tile([128, BT], F32, tag="allred2b")
            for p in range(NPIECE):
                mm_sweep(p, prod)
                if p == NPIECE - 3:
                    # partial Z all-reduce over finished pieces; its ring
                    # latency hides under the last two piece sweeps.
                    for bt in range(BT):
                        nc.vector.reduce_sum(
                            out=payload2[:, bt:bt + 1],
                            in_=zp2d[:, bt, 0:NPIECE - 2],
                            axis=AX.X,
                        )
                    nc.gpsimd.dma_start(out=ccin2[:], in_=payload2[:])
                    nc.gpsimd.collective_compute(
                        "AllReduce", ALU.add,
                        replica_groups=[list(range(NCORES))],
                        ins=[ccin2[:]], outs=[ccout2[:]],
                    )
                    nc.gpsimd.dma_start(out=allred2[:], in_=ccout2[:])
            for _ in prod:
                pass

            # ---------------- all-reduce the last Z partials ----
            for bt in range(BT):
                nc.vector.reduce_sum(
                    out=payload2b[:, bt:bt + 1],
                    in_=zp2d[:, bt, NPIECE - 2:NPIECE],
                    axis=AX.X,
                )
            nc.gpsimd.dma_start(out=ccin3[:], in_=payload2b[:])
            nc.gpsimd.collective_compute(
                "AllReduce", ALU.add,
                replica_groups=[list(range(NCORES))],
                ins=[ccin3[:]], outs=[ccout3[:]],
            )
            nc.gpsimd.dma_start(out=allred2b[:], in_=ccout3[:])
            nc.vector.tensor_tensor(
                out=allred2[:], in0=allred2[:], in1=allred2b[:], op=ALU.add
            )

            # ---------------- final loss math (identical on all cores) ----
            zsum = allred2[:, 0:8]
            rn8 = allred1[:, 8:16]
            zc = s_pool.tile([128, 8], F32, tag="zc")
            lnz = s_pool.tile([128, 8], F32, tag="lnz")
            nll = s_pool.tile([128, 8], F32, tag="nll")
            nc.vector.tensor_scalar_add(
                out=zc[:], in0=zsum, scalar1=-NPAD_TOTAL
            )
            e1, e2 = eref["e1"], eref["e2"]
            nc.vector.tensor_tensor(out=zc[:], in0=zc[:], in1=e1[:], op=ALU.subtract)
            nc.vector.tensor_tensor(out=zc[:], in0=zc[:], in1=e2[:], op=ALU.add)
            nc.scalar.activation(lnz[:], zc[:], AF.Ln)
            # nll = lnz - 64*st + 22.4
            st8 = allred1[:, 0:8]
            nc.vector.scalar_tensor_tensor(
                out=nll[:], in0=st8, scalar=-SCALE, in1=lnz[:],
                op0=ALU.mult, op1=ALU.add,
            )
            nc.vector.tensor_scalar_add(out=nll[:], in0=nll[:], scalar1=SM)
            red2 = s_pool.tile([128, 2], F32, tag="red2")
            nc.vector.reduce_sum(out=red2[:, 0:1], in_=nll[:], axis=AX.X)
            nc.vector.reduce_sum(out=red2[:, 1:2], in_=rn8, axis=AX.X)
            fin_ps = ps_ss.tile([128, 512], F32, tag="ssps")
            nc.tensor.matmul(
                out=fin_ps[0:1, 0:2], lhsT=ones_f32[:], rhs=red2[:],
                start=True, stop=True,
            )
            fin = s_pool.tile([1, 2], F32, tag="fin")
            nc.vector.tensor_copy(out=fin[:], in_=fin_ps[0:1, 0:2])
            p_t = s_pool.tile([1, 1], F32, tag="p_t")
            nc.scalar.activation(p_t[:], fin[:, 0:1], AF.Exp, scale=-1.0 / B)
            q_t = s_pool.tile([1, 1], F32, tag="q_t")
            nc.vector.tensor_scalar(
                out=q_t[:], in0=p_t[:], scalar1=-1.0, scalar2=1.0,
                op0=ALU.mult, op1=ALU.add,
            )
            nc.vector.tensor_tensor(out=q_t[:], in0=q_t[:], in1=q_t[:], op=ALU.mult)
            lgp = s_pool.tile([1, 1], F32, tag="lgp")
            nc.vector.tensor_scalar_mul(out=lgp[:], in0=fin[:, 0:1], scalar1=1.0 / B)
            nc.vector.tensor_tensor(out=q_t[:], in0=q_t[:], in1=lgp[:], op=ALU.mult)
            rterm = s_pool.tile([1, 1], F32, tag="rterm")
            nc.vector.tensor_scalar_mul(
                out=rterm[:], in0=fin[:, 1:2], scalar1=LAMBDA / B
            )
            nc.vector.tensor_tensor(
                out=q_t[:], in0=q_t[:], in1=rterm[:], op=ALU.add
            )
            nc.gpsimd.dma_start(out=out_d[:], in_=q_t[:])

    n = split_multi_waits(nc)
    print(f"split_multi_waits: inserted {n} wait-nops")
    return nc


_NC_CACHE = {}


def _get_nc():
    if "nc" not in _NC_CACHE:
        _NC_CACHE["nc"] = build_bass()
    return _NC_CACHE["nc"]


def make_in_maps(x, target, id_agent, b):
    x = np.ascontiguousarray(np.asarray(x, dtype=np.float32))
    target = np.asarray(target).astype(np.int64)
    id_agent = np.asarray(id_agent, dtype=np.float32)
    b = np.asarray(b, dtype=np.float32)

    bf = ml_dtypes.bfloat16
    x_bf = x.astype(bf)
    xt_bf = np.ascontiguousarray(x_bf.T)

    in_maps = []
    for k in range(NCORES):
        lo = k * CPER
        ia_k = np.zeros((CSH, D), dtype=bf)
        ia_k[:CPER] = id_agent[lo:lo + CPER].astype(bf)
        b_k = np.zeros((CSH, D), dtype=bf)
        b_k[:CPER] = b[lo:lo + CPER].astype(bf)
        iat_k = np.ascontiguousarray(ia_k.T)
        tloc = np.clip(target - lo, 0, CPER - 1).astype(np.int32)
        owned = ((target >= lo) & (target < lo + CPER)).astype(np.float32)
        toff_k = np.ascontiguousarray(tloc.reshape(BT, 128).T)
        tmask_k = np.ascontiguousarray(owned.reshape(BT, 128).T)
        in_maps.append(
            {
                "xbf": x_bf,
                "xtbf": xt_bf,
                "iatbf": iat_k,
                "iabf": ia_k,
                "bbf": b_k,
                "toff": toff_k,
                "tmask": tmask_k,
            }
        )
    return in_maps


def run(inputs, trace=False, **kw):
    nc = _get_nc()
    in_maps = make_in_maps(**inputs)
    res = run_bass_kernel_spmd(
        nc, in_maps, core_ids=list(range(NCORES)), trace=trace, **kw
    )
    return res


def kernel(x, target, id_agent, b):
    res = run({"x": x, "target": target, "id_agent": id_agent, "b": b})
    return np.asarray(res.results[0]["out"], dtype=np.float32)
